# revision 17
# baseline (speedup 1.0000x reference)
"""Trainium2 Bass kernel for nn_NAM_49314814493074 (dense_mlp / NAM).

Computes, for x [B=32768, F=256] and per-feature MLPs (1->8->8->1):
    h1 = relu(x[:,:,None]*W1 + b1);  h2 = relu(einsum(bfh,fhk->bfk, h1, W2) + b2)
    w  = einsum(bfh,fh->bf, h2, W3); output = w.sum(1, keepdims=True)
Returns (output [B,1] f32, w [B,F] f32).

Strategy: pure data-parallel over 8 NeuronCores (4096 batch rows each).
On-device layout is feature-major (xT [F, Bc], bf16). The per-feature MLPs
are evaluated per 16-feature group with three PE matmuls:
  L1 "expand" lhsT [K=128 features, M=128=(16f x 8h)] carrying W1,
  L2 block-diagonal lhsT [(16f x 8h) -> (16f x 8k)] carrying W2,
  L3 lhsT [(16f x 8k) -> 16 w-rows] carrying W3, where group pairs
  (2p, 2p+1) zero-pad to M=32 and accumulate into one 32-row psum block
  (per-element has_written semantics; validated on HW).
relu+bias are fused single DVE/ACT tensor_scalar/activation ops reading
PSUM directly (the PSUM->SBUF move is the relu). The final feature sum is
computed host-side in float64 from the returned w.
"""

import functools
import sys

import numpy as np

B, F, H = 32768, 256, 8
NCORES = 8
BC = B // NCORES          # 4096 batch rows per core
NCHUNK = 512              # batch columns per chunk (= 1 fp32 PSUM bank)
NCH = BC // NCHUNK        # 8 chunks per core
NG = 8                    # 16-feature groups per 128-feature f-tile

for _p in ("/opt/trn_rl_repo", "/root/.axon_site/_ro/trn_rl_repo"):
    if _p not in sys.path:
        sys.path.append(_p)


# ---------------------------------------------------------------- host layout

def _build_weight_arrays(W1, b1, W2, b2, W3):
    """Weight layouts for the group-structured matmuls (natural feature
    order; slot = t*8+g for f-tile t, group g; features f = 128t+16g+s)."""
    t, g, s = np.meshgrid(np.arange(2), np.arange(NG), np.arange(16),
                          indexing="ij")
    f = 128 * t + 16 * g + s
    h = np.arange(H)
    k = np.arange(H)

    L1W = np.zeros((128, 16 * 128), np.float32)
    # slot (t,g): row 16g+s, col 8s+h = W1[f,h]
    r1 = (16 * g + s)[..., None] + 0 * h
    c1 = (128 * (t * NG + g) + 8 * s)[..., None] + h
    L1W[r1.ravel(), c1.ravel()] = W1[f][..., :].ravel()

    L2W = np.zeros((128, 16 * 128), np.float32)
    # slot (t,g): row 8s+h, col 8s+k = W2[f,h,k]
    r2 = (8 * s)[..., None, None] + h[:, None] + 0 * k
    c2 = (128 * (t * NG + g) + 8 * s)[..., None, None] + 0 * h[:, None] + k
    L2W[r2.ravel(), c2.ravel()] = W2[f][..., :, :].ravel()

    L3W = np.zeros((128, 16 * 32), np.float32)
    # slot (t,g): row 8s+k, col 16*(g%2)+s = W3[f,k]; other half zeros
    r3 = (8 * s)[..., None] + k
    c3 = (32 * (t * NG + g) + 16 * (g % 2) + s)[..., None] + 0 * k
    L3W[r3.ravel(), c3.ravel()] = W3[f][..., :].ravel()

    b1A = np.zeros((128, 16), np.float32)
    rb1 = (8 * s)[..., None] + h
    cb1 = (t * NG + g)[..., None] + 0 * h
    b1A[rb1.ravel(), cb1.ravel()] = b1[f][..., :].ravel()

    b2A = np.zeros((128, 16), np.float32)
    rb2 = (8 * s)[..., None] + k
    cb2 = (t * NG + g)[..., None] + 0 * k
    b2A[rb2.ravel(), cb2.ravel()] = b2[f][..., :].ravel()

    return L1W, L2W, L3W, b1A, b2A


# ---------------------------------------------------------------- bass kernel

def _split_waits(nc, max_waits=1):
    """Walrus (this build) rejects instructions with >1 sync-wait command;
    Tile's tail drains can carry 3+. Hoist overflow waits onto NoOps
    inserted immediately before the offending instruction (same engine —
    semantically identical: all waits still precede execution)."""
    import concourse.mybir as mybir

    seen = set()
    k = 0
    for bbw in nc.bb_map.values():
        bb = bbw.bb
        if id(bb) in seen:
            continue
        seen.add(id(bb))
        insts = bb.instructions
        out = []
        changed = False
        for inst in insts:
            si = inst.sync_info
            if si is not None and si.on_wait and len(si.on_wait) > max_waits:
                waits = list(si.on_wait)
                while len(waits) > max_waits:
                    chunk, waits = waits[:max_waits], waits[max_waits:]
                    nop = mybir.InstNoOp(name=f"I-wsplit-{k}", ins=[], outs=[])
                    k += 1
                    nop.engine = inst.engine
                    nop.sync_info = mybir.SyncInfo(on_wait=chunk, on_update=[])
                    out.append(nop)
                inst.sync_info = mybir.SyncInfo(
                    on_wait=waits, on_update=list(si.on_update))
                changed = True
            out.append(inst)
        if changed:
            bb.instructions = out
    return nc


@functools.lru_cache(maxsize=4)
def _build_nc(n_iter=1, nch=NCH, split=True, cfg=()):
    import contextlib

    import concourse.bass as bass
    import concourse.mybir as mybir
    from concourse.tile import TileContext

    f32 = mybir.dt.float32
    bf16 = mybir.dt.bfloat16
    Alu = mybir.AluOpType
    Act = mybir.ActivationFunctionType

    nc = bass.Bass()
    xT = nc.declare_dram_parameter("xT", [F, BC], bf16, isOutput=False)
    L1W = nc.declare_dram_parameter("L1W", [128, 2048], bf16, isOutput=False)
    L2W = nc.declare_dram_parameter("L2W", [128, 2048], bf16, isOutput=False)
    L3W = nc.declare_dram_parameter("L3W", [128, 512], bf16, isOutput=False)
    b1A = nc.declare_dram_parameter("b1A", [128, 16], f32, isOutput=False)
    b2A = nc.declare_dram_parameter("b2A", [128, 16], f32, isOutput=False)
    wT = nc.declare_dram_parameter("wT", [F, BC], f32, isOutput=True)

    cfgd = dict(cfg)
    B_H1P = cfgd.get("h1p", 3)
    B_H2P = cfgd.get("h2p", 3)
    B_PS3 = cfgd.get("ps3", 2)
    B_SB = cfgd.get("sbuf", 4)
    DVE_SHARE = cfgd.get("dve_share", 0.5)  # fraction of relu ops on DVE

    lane = [0.0]  # fractional round-robin accumulator

    with TileContext(nc) as tc:
        with (
            tc.tile_pool(name="const", bufs=1) as cp,
            tc.tile_pool(name="sb", bufs=4) as sb,
            tc.tile_pool(name="ps", bufs=1, space="PSUM") as ps,
        ):
            l1w = cp.tile_from(L1W[:, :])
            l2w = cp.tile_from(L2W[:, :])
            l3w = cp.tile_from(L3W[:, :])
            b1a = cp.tile_from(b1A[:, :])
            b2a = cp.tile_from(b2A[:, :])

            def pick_dve():
                lane[0] += DVE_SHARE
                if lane[0] >= 1.0:
                    lane[0] -= 1.0
                    return True
                return False

            def relu_bias(out, in_, bias_col):
                # out = relu(in_ + bias), reading PSUM, split across engines
                if pick_dve():
                    nc.vector.tensor_scalar(
                        out, in_, bias_col, 0.0, Alu.add, Alu.max)
                else:
                    nc.scalar.activation(out, in_, Act.Relu, bias=bias_col)

            loop_ctx = (tc.For_i(0, n_iter, 1, name="rep")
                        if n_iter > 1 else contextlib.nullcontext())
            with loop_ctx:
                SHIFT = cfgd.get("swp", 1)  # software-pipeline stage shift
                xts = {}     # chunk -> [xt tile t0, t1]
                ps3s = {}    # (chunk, t) -> psum tile
                h1ps, h1ss, h2ps, h2ss = {}, {}, {}, {}

                def load_x(c):
                    cs = slice(c * NCHUNK, (c + 1) * NCHUNK)
                    tiles = []
                    for t in range(2):
                        xt = sb.tile([128, NCHUNK], bf16, tag="xt", bufs=B_SB,
                                     name=f"xt{c}{t}")
                        nc.sync.dma_start(
                            out=xt[:, :], in_=xT[128 * t:128 * (t + 1), cs])
                        tiles.append(xt)
                    xts[c] = tiles

                def st_l1(c, t, g):
                    sl = t * NG + g
                    h1p = ps.tile([128, NCHUNK], f32, tag="h1p", bufs=B_H1P,
                                  name=f"h1p{c}{t}{g}")
                    nc.tensor.matmul(
                        h1p[:, :],
                        lhsT=l1w[:, 128 * sl:128 * sl + 128],
                        rhs=xts[c][t][:, :],
                    )
                    h1ps[c, t, g] = h1p

                def st_r1(c, t, g):
                    sl = t * NG + g
                    h1 = sb.tile([128, NCHUNK], bf16, tag="h1s", bufs=B_SB,
                                 name=f"h1s{c}{t}{g}")
                    relu_bias(h1[:, :], h1ps.pop((c, t, g))[:, :],
                              b1a[:, sl:sl + 1])
                    h1ss[c, t, g] = h1

                def st_l2(c, t, g):
                    sl = t * NG + g
                    h2p = ps.tile([128, NCHUNK], f32, tag="h2p", bufs=B_H2P,
                                  name=f"h2p{c}{t}{g}")
                    nc.tensor.matmul(
                        h2p[:, :],
                        lhsT=l2w[:, 128 * sl:128 * sl + 128],
                        rhs=h1ss.pop((c, t, g))[:, :],
                    )
                    h2ps[c, t, g] = h2p

                def st_r2(c, t, g):
                    sl = t * NG + g
                    h2 = sb.tile([128, NCHUNK], bf16, tag="h2s", bufs=B_SB,
                                 name=f"h2s{c}{t}{g}")
                    relu_bias(h2[:, :], h2ps.pop((c, t, g))[:, :],
                              b2a[:, sl:sl + 1])
                    h2ss[c, t, g] = h2

                def st_l3(c, t, g):
                    sl = t * NG + g
                    if (c, t) not in ps3s:
                        ps3s[c, t] = ps.tile([128, NCHUNK], f32, tag="ps3",
                                             bufs=B_PS3, name=f"ps3{c}{t}")
                    ps3 = ps3s[c, t]
                    # L3: group pairs accumulate into one 32-row block
                    nc.tensor.matmul(
                        ps3[32 * (g // 2):32 * (g // 2) + 32, :],
                        lhsT=l3w[:, 32 * sl:32 * sl + 32],
                        rhs=h2ss.pop((c, t, g))[:, :],
                        start=(g % 2 == 0), stop=(g % 2 == 1),
                        tile_position=(0, 32 * (g // 2)),
                        skip_group_check=True,
                    )
                    if g == NG - 1:
                        cs = slice(c * NCHUNK, (c + 1) * NCHUNK)
                        wts = sb.tile([128, NCHUNK], f32, tag="wts", bufs=3,
                                      name=f"wts{c}{t}")
                        if pick_dve():
                            nc.vector.tensor_copy(wts[:, :], ps3[:, :])
                        else:
                            nc.scalar.copy(wts[:, :], ps3[:, :])
                        nc.sync.dma_start(
                            out=wT[128 * t:128 * (t + 1), cs], in_=wts[:, :])
                        del ps3s[c, t]

                seq = [(c, t, g) for c in range(nch)
                       for g in range(NG) for t in range(2)]
                stages = [st_l1, st_r1, st_l2, st_r2, st_l3]
                n = len(seq)
                load_x(0)
                for i in range(n + 4 * SHIFT):
                    # prefetch next chunk's x mid-way through this chunk
                    if i < n and i % 16 == 8 and (i // 16) + 1 < nch:
                        load_x((i // 16) + 1)
                    for si, stfn in enumerate(stages):
                        idx = i - si * SHIFT
                        if 0 <= idx < n:
                            stfn(*seq[idx])
    return _split_waits(nc) if split else nc


_N_ITER = 1  # repeat count for timing (For_i loop); 1 for grading
_CFG = (("swp", 1),)  # software-pipelined emission for the Tile scheduler

# ---------------------------------------------------------------- entry point


def _run(inputs, trace=False):
    import ml_dtypes

    from concourse.bass_utils import run_bass_kernel_spmd

    bfl = ml_dtypes.bfloat16
    x = np.ascontiguousarray(inputs["input_tensor"], np.float32)
    W1 = np.asarray(inputs["W1"], np.float32)
    b1 = np.asarray(inputs["b1"], np.float32)
    W2 = np.asarray(inputs["W2"], np.float32)
    b2 = np.asarray(inputs["b2"], np.float32)
    W3 = np.asarray(inputs["W3"], np.float32)

    L1Wa, L2Wa, L3Wa, b1Aa, b2Aa = _build_weight_arrays(W1, b1, W2, b2, W3)

    nc = _build_nc(_N_ITER, NCH, True, _CFG)
    xTp = x.T.astype(bfl)  # [F, B] bf16
    in_maps = []
    for core in range(NCORES):
        shard = np.ascontiguousarray(xTp[:, core * BC:(core + 1) * BC])
        in_maps.append({
            "xT": shard, "L1W": L1Wa.astype(bfl), "L2W": L2Wa.astype(bfl),
            "L3W": L3Wa.astype(bfl), "b1A": b1Aa, "b2A": b2Aa,
        })
    res = run_bass_kernel_spmd(nc, in_maps, list(range(NCORES)), trace=trace)
    wT_full = np.concatenate([r["wT"] for r in res.results], axis=1)  # [F, B]
    w = np.ascontiguousarray(wT_full.T)
    out = w.sum(axis=1, keepdims=True, dtype=np.float64).astype(np.float32)
    return (out, w), res


def kernel(**inputs):
    outs, _ = _run(inputs, trace=False)
    return outs


if __name__ == "__main__":
    rng = np.random.default_rng(0)
    demo = {
        "input_tensor": rng.standard_normal((B, F)).astype(np.float32),
        "W1": rng.standard_normal((F, H)).astype(np.float32) * 0.5,
        "b1": rng.standard_normal((F, H)).astype(np.float32) * 0.1,
        "W2": rng.standard_normal((F, H, H)).astype(np.float32) / np.sqrt(H),
        "b2": rng.standard_normal((F, H)).astype(np.float32) * 0.1,
        "W3": rng.standard_normal((F, H)).astype(np.float32) / np.sqrt(H),
    }
    out, w = kernel(**demo)
    print(out.shape, w.shape, float(out[0, 0]))
